# revision 19
# baseline (speedup 1.0000x reference)
"""Trainium2 Bass kernel for the masked per-site stencil contraction

    y[o, n] = f( sum_{i,k} Wconv[o,i,k] * mask[n,o,i,k] * x[i, shifts[n,k]] + bconv[o] )
    f(v) = (sigmoid(v) - 0.5) * (2 + 2e)/(e - 1) = (2+2e)/(2(e-1)) * tanh(v/2)

Shapes: O=I=32, K=13, N=4096.  Sharded over 8 NeuronCores along the site
dimension N (512 sites per core); mask / shifts / output columns are
partitioned, x / Wconv / bconv replicated.

Per-core device plan (all cores run the identical SPMD program):
  * g built by 13 dma_gather calls (one per tap k) from xT4 in HBM, where
    xT4[s, 32a+i] = x[i, s] (x^T replicated 4x along features, 256B rows).
    transpose=True lands the feature dim on partitions: g_k[32a+i, n] =
    x[i, shifts[n, k]].  SWDGE descriptors prep on GPSIMD, data moves on
    the 16 DMA engines across 4 SWDGE queues.
  * mask shipped as fp16 (exact for a 0/1 mask) in [og, k, (j,i), n]
    layout: 4 output channels j packed along the 128-partition dim.
  * DVE: prod[(j,i), n] = mask_tile[og, k] * g_k  (fp16, 2x_1P mode)
  * PE:  one m=4 matmul per (og, k): ypsum[4og:4og+4, n] += W4^T @ prod,
    13-long accumulation chains per og, all 8 chains in ONE PSUM bank
    ([32, 512] f32).  (og, k) emission follows a diagonal sort matching
    DMA/gather arrival order.
  * ACT: single tanh over [32, 512] PSUM with per-partition bias; DVE
    scale; one output DMA.
"""

import math

import numpy as np

import concourse.bacc as bacc
import concourse.mybir as mybir
from concourse import tile
from concourse.bass_utils import run_bass_kernel_spmd

O, I, K, N = 32, 32, 13, 4096
NCORES = 8
NS = N // NCORES          # 512 local sites per core
NOG = O // 4              # 8 channel groups of 4
_E = math.e
SCALE = (2.0 + 2.0 * _E) / (_E - 1.0)

_F32 = mybir.dt.float32
_F16 = mybir.dt.float16
_I16 = mybir.dt.int16

_BUILT = {}


def _emit(nc, tc, d, pools):
    cpool, gpool, mpool, ppool, opool, qpool = pools

    idx_sb = cpool.tile([128, K * 32], _I16, tag="idx")
    nc.sync.dma_start(idx_sb[:, :], d["idx"][:, :])
    wt4f = cpool.tile([128, K, NOG, 4], _F32, tag="w4f")
    nc.scalar.dma_start(wt4f[:, :, :, :], d["wt4"][:, :, :, :])
    bcol_sb = cpool.tile([4, NOG], _F32, tag="bc")
    nc.scalar.dma_start(bcol_sb[:, :], d["bcol"][:, :])

    # g[32a+i, k, 0, n] = x[i, shifts[n, k]] via SWDGE gather from xT4 rows
    g = gpool.tile([128, K, 1, NS], _F16, tag="g")
    for k in range(K):
        nc.gpsimd.dma_gather(
            g[:, k, :, :],
            d["xT4"][:, :],
            idx_sb[:, 32 * k : 32 * (k + 1)],
            num_idxs=NS,
            num_idxs_reg=NS,
            elem_size=128,
            transpose=True,
            queue_num=k % 4,
        )

    # all 8 og mask tiles stay resident (13.3 KB/partition each); p-major
    # HBM layout means one 2-level-AP DMA per og with 13.3 KB contiguous
    # per-partition lines.
    mts = []
    for og in range(NOG):
        mt = mpool.tile([128, K, NS], _F16, tag=f"m{og}", bufs=1, name=f"mt{og}")
        eng = nc.sync if og % 2 == 0 else nc.scalar
        eng.dma_start(mt[:, :, :], d["maskg"][og])
        mts.append(mt)

    wt4 = cpool.tile([128, K, NOG, 4], _F16, tag="w4")
    nc.vector.tensor_copy(wt4[:, :, :, :], wt4f[:, :, :, :])
    bhalf = opool.tile([4, NOG], _F32, tag="bh")
    nc.scalar.activation(
        bhalf[:, :], bcol_sb[:, :], mybir.ActivationFunctionType.Copy, scale=0.5
    )

    # one PSUM bank per og chain: [4, 512] f32 at base partition 0
    yps = [
        qpool.tile([4, NS], _F32, tag=f"yp{og}", bufs=1, name=f"yp{og}")
        for og in range(NOG)
    ]

    # PE warm-up: a burst of throwaway matmuls gated only on a local memset,
    # so they run during the SWDGE-init dead time and lift the HAM clock to
    # 2.4 GHz before the real accumulation chains arrive.
    warmsrc = cpool.tile([128, NS], _F16, tag="wsrc")
    nc.vector.memset(warmsrc[:, :], 0.0)
    for w in range(24):
        nc.tensor.matmul(
            yps[0][:, :], warmsrc[:, 0:4], warmsrc[:, :], start=True, stop=True
        )

    # k-major emission matches gather arrival order; mask og tiles are all
    # resident well before their first use.
    order = [(og, k) for k in range(K) for og in range(NOG)]
    for og, k in order:
        prod = ppool.tile([128, NS], _F16, tag="prod", bufs=6)
        nc.vector.tensor_mul(prod[:, :], mts[og][:, k, :], g[:, k, 0, :])
        nc.tensor.matmul(
            yps[og][:, :],
            wt4[:, k, og, :],
            prod[:, :],
            start=(k == 0),
            stop=(k == K - 1),
        )
        if k == K - 1:
            ycat = opool.tile([4, NS], _F32, tag="ycat", bufs=2)
            nc.scalar.activation(
                ycat[:, :], yps[og][:, :], mybir.ActivationFunctionType.Tanh,
                bias=bhalf[:, og : og + 1], scale=0.5,
            )
            nc.vector.tensor_scalar_mul(ycat[:, :], ycat[:, :], SCALE / 2.0)
            eng = nc.sync if og % 2 == 0 else nc.scalar
            eng.dma_start(d["y"][4 * og : 4 * og + 4, :], ycat[:, :])


def _declare(nc):
    d = {}
    d["xT4"] = nc.declare_dram_parameter("xT4", [N, 128], _F16, isOutput=False)
    d["idx"] = nc.declare_dram_parameter("idx", [128, K * 32], _I16, isOutput=False)
    d["maskg"] = nc.declare_dram_parameter(
        "maskg", [NOG, 128, K, NS], _F16, isOutput=False
    )
    d["wt4"] = nc.declare_dram_parameter("wt4", [128, K, NOG, 4], _F32, isOutput=False)
    d["bcol"] = nc.declare_dram_parameter("bcol", [4, NOG], _F32, isOutput=False)
    d["y"] = nc.declare_dram_parameter("y", [O, NS], _F32, isOutput=True)
    return d


def _pools(tc, stack):
    names = [
        ("const", 1), ("gather", 1), ("mask", 8), ("prod", 6),
        ("out", 1), ("psum", 1),
    ]
    pools = []
    for name, bufs in names:
        kw = {"space": "PSUM"} if name == "psum" else {}
        pools.append(stack.enter_context(tc.tile_pool(name=name, bufs=bufs, **kw)))
    return pools


def _build():
    if "nc" in _BUILT:
        return _BUILT["nc"]
    from contextlib import ExitStack

    nc = bacc.Bacc(
        "TRN2", target_bir_lowering=False, debug=False, num_swdge_queues=4
    )
    d = _declare(nc)
    with tile.TileContext(nc) as tc:
        with ExitStack() as stack:
            pools = _pools(tc, stack)
            _emit(nc, tc, d, pools)
    nc.compile()
    _BUILT["nc"] = nc
    return nc


def _wrap16(col):
    """shifts column (NS,) -> (16, NS//16) wrapped layout: out[r, s] = col[s*16+r]."""
    return np.ascontiguousarray(col.reshape(NS // 16, 16).T)


def make_in_maps(x, Wconv, bconv, mask, shifts):
    """Host-side shard/layout prep: transposes/reshapes plus value-preserving
    dtype narrowing (f32 0/1 mask -> fp16 exactly; int32 indices < 4096 ->
    int16)."""
    x = np.asarray(x, dtype=np.float32)
    xT4 = np.ascontiguousarray(np.tile(x.T.astype(np.float16), (1, 4)))  # (N, 128)
    W = np.asarray(Wconv, dtype=np.float32)
    W4d = W.reshape(NOG, 4, I, K)                                # (og, j, i, k)
    wt4 = np.zeros((128, K, NOG, 4), np.float32)
    for j in range(4):
        wt4[32 * j : 32 * (j + 1), :, :, j] = W4d[:, j].transpose(1, 2, 0)
    bcol = np.ascontiguousarray(
        np.asarray(bconv, dtype=np.float32).reshape(NOG, 4).T
    )                                                            # (4, og)
    mask = np.asarray(mask)
    shifts = np.asarray(shifts)

    in_maps = []
    for core in range(NCORES):
        sl = slice(core * NS, (core + 1) * NS)
        m = mask[sl].astype(np.float16)                          # (NS, O, I, K)
        m = m.transpose(1, 2, 3, 0)                              # (O, I, K, NS)
        maskg = np.ascontiguousarray(m.reshape(NOG, 128, K, NS))
        sh = shifts[sl].astype(np.int16)                         # (NS, K)
        idx = np.empty((128, K * 32), np.int16)
        for k in range(K):
            w = _wrap16(sh[:, k])                                # (16, 32)
            for a in range(8):
                idx[16 * a : 16 * (a + 1), 32 * k : 32 * (k + 1)] = w
        in_maps.append(
            {"xT4": xT4, "idx": idx, "maskg": maskg, "wt4": wt4, "bcol": bcol}
        )
    return in_maps


def kernel(x, Wconv, bconv, mask, shifts):
    nc = _build()
    in_maps = make_in_maps(x, Wconv, bconv, mask, shifts)
    res = run_bass_kernel_spmd(nc, in_maps, core_ids=list(range(NCORES)))
    y = np.empty((O, N), np.float32)
    for core in range(NCORES):
        y[:, core * NS : (core + 1) * NS] = res.results[core]["y"]
    return y


# revision 20
# speedup vs baseline: 1.0460x; 1.0460x over previous
"""Trainium2 Bass kernel for the masked per-site stencil contraction

    y[o, n] = f( sum_{i,k} Wconv[o,i,k] * mask[n,o,i,k] * x[i, shifts[n,k]] + bconv[o] )
    f(v) = (sigmoid(v) - 0.5) * (2 + 2e)/(e - 1) = (2+2e)/(2(e-1)) * tanh(v/2)

Shapes: O=I=32, K=13, N=4096.  Sharded over 8 NeuronCores along the site
dimension N (512 sites per core); mask / shifts / output columns are
partitioned, x / Wconv / bconv replicated.

Per-core device plan (all cores run the identical SPMD program):
  * g built by 13 dma_gather calls (one per tap k) from xT4 in HBM, where
    xT4[s, 32a+i] = x[i, s] (x^T replicated 4x along features, 256B rows).
    transpose=True lands the feature dim on partitions: g_k[32a+i, n] =
    x[i, shifts[n, k]].  SWDGE descriptors prep on GPSIMD, data moves on
    the 16 DMA engines across 4 SWDGE queues.
  * mask shipped as fp16 (exact for a 0/1 mask) in [og, k, (j,i), n]
    layout: 4 output channels j packed along the 128-partition dim.
  * DVE: prod[(j,i), n] = mask_tile[og, k] * g_k  (fp16, 2x_1P mode)
  * PE:  one m=4 matmul per (og, k): ypsum[4og:4og+4, n] += W4^T @ prod,
    13-long accumulation chains per og, all 8 chains in ONE PSUM bank
    ([32, 512] f32).  (og, k) emission follows a diagonal sort matching
    DMA/gather arrival order.
  * ACT: single tanh over [32, 512] PSUM with per-partition bias; DVE
    scale; one output DMA.
"""

import math

import numpy as np

import concourse.bacc as bacc
import concourse.mybir as mybir
from concourse import tile
from concourse.bass_utils import run_bass_kernel_spmd

O, I, K, N = 32, 32, 13, 4096
NCORES = 8
NS = N // NCORES          # 512 local sites per core
NOG = O // 4              # 8 channel groups of 4
_E = math.e
SCALE = (2.0 + 2.0 * _E) / (_E - 1.0)

_F32 = mybir.dt.float32
_F16 = mybir.dt.float16
_I16 = mybir.dt.int16

_BUILT = {}


def _emit(nc, tc, d, pools):
    cpool, gpool, mpool, ppool, opool, qpool = pools

    idx_sb = cpool.tile([128, K * 32], _I16, tag="idx")
    nc.sync.dma_start(idx_sb[:, :], d["idx"][:, :])
    wt4f = cpool.tile([128, K, NOG, 4], _F32, tag="w4f")
    nc.scalar.dma_start(wt4f[:, :, :, :], d["wt4"][:, :, :, :])
    bcol_sb = cpool.tile([4, NOG], _F32, tag="bc")
    nc.scalar.dma_start(bcol_sb[:, :], d["bcol"][:, :])

    # g[32a+i, k, 0, n] = x[i, shifts[n, k]] via SWDGE gather from xT4 rows
    g = gpool.tile([128, K, 1, NS], _F16, tag="g")
    for k in range(K):
        nc.gpsimd.dma_gather(
            g[:, k, :, :],
            d["xT4"][:, :],
            idx_sb[:, 32 * k : 32 * (k + 1)],
            num_idxs=NS,
            num_idxs_reg=NS,
            elem_size=128,
            transpose=True,
            queue_num=k % 4,
        )

    # all 8 og mask tiles stay resident (13.3 KB/partition each); p-major
    # HBM layout means one 2-level-AP DMA per og with 13.3 KB contiguous
    # per-partition lines.
    mts = []
    for og in range(NOG):
        mt = mpool.tile([128, K, NS], _F16, tag=f"m{og}", bufs=1, name=f"mt{og}")
        eng = nc.sync if og % 2 == 0 else nc.scalar
        eng.dma_start(mt[:, :, :], d["maskg"][og])
        mts.append(mt)

    wt4 = cpool.tile([128, K, NOG, 4], _F16, tag="w4")
    nc.vector.tensor_copy(wt4[:, :, :, :], wt4f[:, :, :, :])
    bhalf = opool.tile([4, NOG], _F32, tag="bh")
    nc.scalar.activation(
        bhalf[:, :], bcol_sb[:, :], mybir.ActivationFunctionType.Copy, scale=0.5
    )

    # one PSUM bank per og chain: [4, 512] f32 at base partition 0
    yps = [
        qpool.tile([4, NS], _F32, tag=f"yp{og}", bufs=1, name=f"yp{og}")
        for og in range(NOG)
    ]

    # PE warm-up: throwaway matmuls keep the HAM clock at 2.4 GHz through
    # the SWDGE-init dead time.  An initial burst (gated on a local memset)
    # plus batches gated on successive mask-tile arrivals to avoid the
    # idle-window re-throttle before the first products appear.
    warmsrc = cpool.tile([128, NS], _F16, tag="wsrc")
    nc.vector.memset(warmsrc[:, :], 0.0)
    for w in range(16):
        nc.tensor.matmul(
            yps[0][:, :], warmsrc[:, 0:4], warmsrc[:, :], start=True, stop=True
        )
    for og in range(4):
        for w in range(6):
            nc.tensor.matmul(
                yps[0][:, :], wt4[:, 0, og, :], mts[og][:, 0, :],
                start=True, stop=True,
            )

    # k-major emission matches gather arrival order; DVE muls batch tap
    # pairs (adjacent in both the mask tile and the gather tile) to halve
    # instruction overhead on the critical DVE->PE lockstep.
    kbatches = [(0, 1), (2, 3), (4, 5), (6, 7), (8, 9), (10, 11), (12,)]
    for kb in kbatches:
        for og in range(NOG):
            k0, nb = kb[0], len(kb)
            prod = ppool.tile([128, nb, NS], _F16, tag=f"prod{nb}", bufs=6)
            nc.vector.tensor_mul(
                prod[:, :, :],
                mts[og][:, k0 : k0 + nb, :],
                g[:, k0 : k0 + nb, 0, :],
            )
            for k in kb:
                nc.tensor.matmul(
                    yps[og][:, :],
                    wt4[:, k, og, :],
                    prod[:, k - k0, :],
                    start=(k == 0),
                    stop=(k == K - 1),
                )
            if kb[-1] == K - 1:
                ycat = opool.tile([4, NS], _F32, tag="ycat", bufs=2)
                nc.scalar.activation(
                    ycat[:, :], yps[og][:, :], mybir.ActivationFunctionType.Tanh,
                    bias=bhalf[:, og : og + 1], scale=0.5,
                )
                nc.vector.tensor_scalar_mul(ycat[:, :], ycat[:, :], SCALE / 2.0)
                eng = nc.sync if og % 2 == 0 else nc.scalar
                eng.dma_start(d["y"][4 * og : 4 * og + 4, :], ycat[:, :])


def _declare(nc):
    d = {}
    d["xT4"] = nc.declare_dram_parameter("xT4", [N, 128], _F16, isOutput=False)
    d["idx"] = nc.declare_dram_parameter("idx", [128, K * 32], _I16, isOutput=False)
    d["maskg"] = nc.declare_dram_parameter(
        "maskg", [NOG, 128, K, NS], _F16, isOutput=False
    )
    d["wt4"] = nc.declare_dram_parameter("wt4", [128, K, NOG, 4], _F32, isOutput=False)
    d["bcol"] = nc.declare_dram_parameter("bcol", [4, NOG], _F32, isOutput=False)
    d["y"] = nc.declare_dram_parameter("y", [O, NS], _F32, isOutput=True)
    return d


def _pools(tc, stack):
    names = [
        ("const", 1), ("gather", 1), ("mask", 8), ("prod", 6),
        ("out", 1), ("psum", 1),
    ]
    pools = []
    for name, bufs in names:
        kw = {"space": "PSUM"} if name == "psum" else {}
        pools.append(stack.enter_context(tc.tile_pool(name=name, bufs=bufs, **kw)))
    return pools


def _build():
    if "nc" in _BUILT:
        return _BUILT["nc"]
    from contextlib import ExitStack

    nc = bacc.Bacc(
        "TRN2", target_bir_lowering=False, debug=False, num_swdge_queues=4
    )
    d = _declare(nc)
    with tile.TileContext(nc) as tc:
        with ExitStack() as stack:
            pools = _pools(tc, stack)
            _emit(nc, tc, d, pools)
    nc.compile()
    _BUILT["nc"] = nc
    return nc


def _wrap16(col):
    """shifts column (NS,) -> (16, NS//16) wrapped layout: out[r, s] = col[s*16+r]."""
    return np.ascontiguousarray(col.reshape(NS // 16, 16).T)


def make_in_maps(x, Wconv, bconv, mask, shifts):
    """Host-side shard/layout prep: transposes/reshapes plus value-preserving
    dtype narrowing (f32 0/1 mask -> fp16 exactly; int32 indices < 4096 ->
    int16)."""
    x = np.asarray(x, dtype=np.float32)
    xT4 = np.ascontiguousarray(np.tile(x.T.astype(np.float16), (1, 4)))  # (N, 128)
    W = np.asarray(Wconv, dtype=np.float32)
    W4d = W.reshape(NOG, 4, I, K)                                # (og, j, i, k)
    wt4 = np.zeros((128, K, NOG, 4), np.float32)
    for j in range(4):
        wt4[32 * j : 32 * (j + 1), :, :, j] = W4d[:, j].transpose(1, 2, 0)
    bcol = np.ascontiguousarray(
        np.asarray(bconv, dtype=np.float32).reshape(NOG, 4).T
    )                                                            # (4, og)
    mask = np.asarray(mask)
    shifts = np.asarray(shifts)

    in_maps = []
    for core in range(NCORES):
        sl = slice(core * NS, (core + 1) * NS)
        m = mask[sl].astype(np.float16)                          # (NS, O, I, K)
        m = m.transpose(1, 2, 3, 0)                              # (O, I, K, NS)
        maskg = np.ascontiguousarray(m.reshape(NOG, 128, K, NS))
        sh = shifts[sl].astype(np.int16)                         # (NS, K)
        idx = np.empty((128, K * 32), np.int16)
        for k in range(K):
            w = _wrap16(sh[:, k])                                # (16, 32)
            for a in range(8):
                idx[16 * a : 16 * (a + 1), 32 * k : 32 * (k + 1)] = w
        in_maps.append(
            {"xT4": xT4, "idx": idx, "maskg": maskg, "wt4": wt4, "bcol": bcol}
        )
    return in_maps


def kernel(x, Wconv, bconv, mask, shifts):
    nc = _build()
    in_maps = make_in_maps(x, Wconv, bconv, mask, shifts)
    res = run_bass_kernel_spmd(nc, in_maps, core_ids=list(range(NCORES)))
    y = np.empty((O, N), np.float32)
    for core in range(NCORES):
        y[:, core * NS : (core + 1) * NS] = res.results[core]["y"]
    return y


# revision 23
# speedup vs baseline: 1.0534x; 1.0071x over previous
"""Trainium2 Bass kernel for the masked per-site stencil contraction

    y[o, n] = f( sum_{i,k} Wconv[o,i,k] * mask[n,o,i,k] * x[i, shifts[n,k]] + bconv[o] )
    f(v) = (sigmoid(v) - 0.5) * (2 + 2e)/(e - 1) = (2+2e)/(2(e-1)) * tanh(v/2)

Shapes: O=I=32, K=13, N=4096.  Sharded over 8 NeuronCores along the site
dimension N (512 sites per core); mask / shifts / output columns are
partitioned, x / Wconv / bconv replicated.

Per-core device plan (all cores run the identical SPMD program):
  * g built by 13 dma_gather calls (one per tap k) from xT4 in HBM, where
    xT4[s, 32a+i] = x[i, s] (x^T replicated 4x along features, 256B rows).
    transpose=True lands the feature dim on partitions: g_k[32a+i, n] =
    x[i, shifts[n, k]].  SWDGE descriptors prep on GPSIMD, data moves on
    the 16 DMA engines across 4 SWDGE queues.
  * mask shipped as fp16 (exact for a 0/1 mask) in [og, k, (j,i), n]
    layout: 4 output channels j packed along the 128-partition dim.
  * DVE: prod[(j,i), n] = mask_tile[og, k] * g_k  (fp16, 2x_1P mode)
  * PE:  one m=4 matmul per (og, k): ypsum[4og:4og+4, n] += W4^T @ prod,
    13-long accumulation chains per og, all 8 chains in ONE PSUM bank
    ([32, 512] f32).  (og, k) emission follows a diagonal sort matching
    DMA/gather arrival order.
  * ACT: single tanh over [32, 512] PSUM with per-partition bias; DVE
    scale; one output DMA.
"""

import math

import numpy as np

import concourse.bacc as bacc
import concourse.mybir as mybir
from concourse import tile
from concourse.bass_utils import run_bass_kernel_spmd

O, I, K, N = 32, 32, 13, 4096
NCORES = 8
NS = N // NCORES          # 512 local sites per core
NOG = O // 4              # 8 channel groups of 4
_E = math.e
SCALE = (2.0 + 2.0 * _E) / (_E - 1.0)

_F32 = mybir.dt.float32
_F16 = mybir.dt.float16
_I16 = mybir.dt.int16

_BUILT = {}


def _emit(nc, tc, d, pools):
    cpool, gpool, mpool, ppool, opool, qpool = pools

    idx_sb = cpool.tile([128, K * 32], _I16, tag="idx")
    nc.sync.dma_start(idx_sb[:, :], d["idx"][:, :])
    wt4f = cpool.tile([128, K, NOG, 4], _F32, tag="w4f")
    nc.scalar.dma_start(wt4f[:, :, :, :], d["wt4"][:, :, :, :])
    bcol_sb = cpool.tile([4, NOG], _F32, tag="bc")
    nc.scalar.dma_start(bcol_sb[:, :], d["bcol"][:, :])

    # g[32a+i, k, 0, n] = x[i, shifts[n, k]] via SWDGE gather from xT4 rows
    g = gpool.tile([128, K, 1, NS], _F16, tag="g")
    for k in range(K):
        nc.gpsimd.dma_gather(
            g[:, k, :, :],
            d["xT4"][:, :],
            idx_sb[:, 32 * k : 32 * (k + 1)],
            num_idxs=NS,
            num_idxs_reg=NS,
            elem_size=128,
            transpose=True,
            queue_num=k % 4,
        )

    # all 8 og mask tiles stay resident (13.3 KB/partition each); p-major
    # HBM layout gives 2-level-AP DMAs with contiguous per-partition lines.
    # Each og is split into two k-halves, all h0 halves DMA'd before any h1,
    # so every og's early taps are resident by the time its products start.
    KH = 7
    mts = []
    for og in range(NOG):
        mt = mpool.tile([128, K, NS], _F16, tag=f"m{og}", bufs=1, name=f"mt{og}")
        eng = nc.sync if og % 2 == 0 else nc.scalar
        eng.dma_start(mt[:, :, :], d["maskg"][og])
        mts.append(mt)

    wt4 = cpool.tile([128, K, NOG, 4], _F16, tag="w4")
    nc.vector.tensor_copy(wt4[:, :, :, :], wt4f[:, :, :, :])
    bhalf = opool.tile([4, NOG], _F32, tag="bh")
    nc.scalar.activation(
        bhalf[:, :], bcol_sb[:, :], mybir.ActivationFunctionType.Copy, scale=0.5
    )

    # one PSUM bank per og chain: [4, 512] f32 at base partition 0
    yps = [
        qpool.tile([4, NS], _F32, tag=f"yp{og}", bufs=1, name=f"yp{og}")
        for og in range(NOG)
    ]

    # PE warm-up: throwaway matmuls keep the HAM clock at 2.4 GHz through
    # the SWDGE-init dead time.  An initial burst (gated on a local memset)
    # plus batches gated on successive mask-tile arrivals to avoid the
    # idle-window re-throttle before the first products appear.
    warmsrc = cpool.tile([128, NS], _F16, tag="wsrc")
    nc.vector.memset(warmsrc[:, :], 0.0)
    for w in range(16):
        nc.tensor.matmul(
            yps[0][:, :], warmsrc[:, 0:4], warmsrc[:, :], start=True, stop=True
        )
    for og in range(4):
        for w in range(6):
            nc.tensor.matmul(
                yps[0][:, :], wt4[:, 0, og, :], mts[og][:, 0, :],
                start=True, stop=True,
            )

    # Emission follows expected arrival: gathers land ~26+1.5k us, mask
    # halves land ~8/~28 + 4.6*(og//2) us; sort (kb, og) by the later of
    # the two.  DVE muls batch tap pairs (adjacent in both the mask tile
    # and the gather tile) to halve instruction overhead on the critical
    # DVE->PE lockstep.
    kbatches = [(0, 1), (2, 3), (4, 5), (6, 7), (8, 9), (10, 11), (12,)]

    def _key(kb, og):
        tg = 26.0 + 1.5 * kb[-1]
        tm = (8.0 if kb[-1] < KH else 28.0) + 4.6 * (og // 2)
        return (max(tg, tm), kb[-1], og)

    order = sorted(
        ((kb, og) for kb in kbatches for og in range(NOG)),
        key=lambda t: _key(t[0], t[1]),
    )
    for kb, og in order:
            k0, nb = kb[0], len(kb)
            prod = ppool.tile([128, nb, NS], _F16, tag=f"prod{nb}", bufs=6)
            nc.vector.tensor_mul(
                prod[:, :, :],
                mts[og][:, k0 : k0 + nb, :],
                g[:, k0 : k0 + nb, 0, :],
            )
            for k in kb:
                nc.tensor.matmul(
                    yps[og][:, :],
                    wt4[:, k, og, :],
                    prod[:, k - k0, :],
                    start=(k == 0),
                    stop=(k == K - 1),
                )
            if kb[-1] == K - 1:
                ycat = opool.tile([4, NS], _F32, tag="ycat", bufs=2)
                nc.scalar.activation(
                    ycat[:, :], yps[og][:, :], mybir.ActivationFunctionType.Tanh,
                    bias=bhalf[:, og : og + 1], scale=0.5,
                )
                nc.vector.tensor_scalar_mul(ycat[:, :], ycat[:, :], SCALE / 2.0)
                eng = nc.sync if og % 2 == 0 else nc.scalar
                eng.dma_start(d["y"][4 * og : 4 * og + 4, :], ycat[:, :])


def _declare(nc):
    d = {}
    d["xT4"] = nc.declare_dram_parameter("xT4", [N, 128], _F16, isOutput=False)
    d["idx"] = nc.declare_dram_parameter("idx", [128, K * 32], _I16, isOutput=False)
    d["maskg"] = nc.declare_dram_parameter(
        "maskg", [NOG, 128, K, NS], _F16, isOutput=False
    )
    d["wt4"] = nc.declare_dram_parameter("wt4", [128, K, NOG, 4], _F32, isOutput=False)
    d["bcol"] = nc.declare_dram_parameter("bcol", [4, NOG], _F32, isOutput=False)
    d["y"] = nc.declare_dram_parameter("y", [O, NS], _F32, isOutput=True)
    return d


def _pools(tc, stack):
    names = [
        ("const", 1), ("gather", 1), ("mask", 8), ("prod", 6),
        ("out", 1), ("psum", 1),
    ]
    pools = []
    for name, bufs in names:
        kw = {"space": "PSUM"} if name == "psum" else {}
        pools.append(stack.enter_context(tc.tile_pool(name=name, bufs=bufs, **kw)))
    return pools


def _build():
    if "nc" in _BUILT:
        return _BUILT["nc"]
    from contextlib import ExitStack

    nc = bacc.Bacc(
        "TRN2", target_bir_lowering=False, debug=False, num_swdge_queues=4
    )
    d = _declare(nc)
    with tile.TileContext(nc) as tc:
        with ExitStack() as stack:
            pools = _pools(tc, stack)
            _emit(nc, tc, d, pools)
    nc.compile()
    _BUILT["nc"] = nc
    return nc


def _wrap16(col):
    """shifts column (NS,) -> (16, NS//16) wrapped layout: out[r, s] = col[s*16+r]."""
    return np.ascontiguousarray(col.reshape(NS // 16, 16).T)


def make_in_maps(x, Wconv, bconv, mask, shifts):
    """Host-side shard/layout prep: transposes/reshapes plus value-preserving
    dtype narrowing (f32 0/1 mask -> fp16 exactly; int32 indices < 4096 ->
    int16)."""
    x = np.asarray(x, dtype=np.float32)
    xT4 = np.ascontiguousarray(np.tile(x.T.astype(np.float16), (1, 4)))  # (N, 128)
    W = np.asarray(Wconv, dtype=np.float32)
    W4d = W.reshape(NOG, 4, I, K)                                # (og, j, i, k)
    wt4 = np.zeros((128, K, NOG, 4), np.float32)
    for j in range(4):
        wt4[32 * j : 32 * (j + 1), :, :, j] = W4d[:, j].transpose(1, 2, 0)
    bcol = np.ascontiguousarray(
        np.asarray(bconv, dtype=np.float32).reshape(NOG, 4).T
    )                                                            # (4, og)
    mask = np.asarray(mask)
    shifts = np.asarray(shifts)

    in_maps = []
    for core in range(NCORES):
        sl = slice(core * NS, (core + 1) * NS)
        m = mask[sl].astype(np.float16)                          # (NS, O, I, K)
        m = m.transpose(1, 2, 3, 0)                              # (O, I, K, NS)
        maskg = np.ascontiguousarray(m.reshape(NOG, 128, K, NS))
        sh = shifts[sl].astype(np.int16)                         # (NS, K)
        idx = np.empty((128, K * 32), np.int16)
        for k in range(K):
            w = _wrap16(sh[:, k])                                # (16, 32)
            for a in range(8):
                idx[16 * a : 16 * (a + 1), 32 * k : 32 * (k + 1)] = w
        in_maps.append(
            {"xT4": xT4, "idx": idx, "maskg": maskg, "wt4": wt4, "bcol": bcol}
        )
    return in_maps


def kernel(x, Wconv, bconv, mask, shifts):
    nc = _build()
    in_maps = make_in_maps(x, Wconv, bconv, mask, shifts)
    res = run_bass_kernel_spmd(nc, in_maps, core_ids=list(range(NCORES)))
    y = np.empty((O, N), np.float32)
    for core in range(NCORES):
        y[:, core * NS : (core + 1) * NS] = res.results[core]["y"]
    return y


# revision 28
# speedup vs baseline: 1.2673x; 1.2030x over previous
"""Trainium2 Bass kernel for the masked per-site stencil contraction

    y[o, n] = f( sum_{i,k} Wconv[o,i,k] * mask[n,o,i,k] * x[i, shifts[n,k]] + bconv[o] )
    f(v) = (sigmoid(v) - 0.5) * (2 + 2e)/(e - 1) = (2+2e)/(2(e-1)) * tanh(v/2)

Shapes: O=I=32, K=13, N=4096.  Sharded over 8 NeuronCores along the site
dimension N (512 sites per core); mask / shifts / output columns are
partitioned, x / Wconv / bconv replicated.

Per-core device plan (all cores run the identical SPMD program):
  * g built by 13 dma_gather calls (one per tap k) from xT4 in HBM, where
    xT4[s, 32a+i] = x[i, s] (x^T replicated 4x along features, 256B rows).
    transpose=True lands the feature dim on partitions: g_k[32a+i, n] =
    x[i, shifts[n, k]].  SWDGE descriptors prep on GPSIMD, data moves on
    the 16 DMA engines across 4 SWDGE queues.
  * mask shipped as fp16 (exact for a 0/1 mask) in [og, k, (j,i), n]
    layout: 4 output channels j packed along the 128-partition dim.
  * DVE: prod[(j,i), n] = mask_tile[og, k] * g_k  (fp16, 2x_1P mode)
  * PE:  one m=4 matmul per (og, k): ypsum[4og:4og+4, n] += W4^T @ prod,
    13-long accumulation chains per og, all 8 chains in ONE PSUM bank
    ([32, 512] f32).  (og, k) emission follows a diagonal sort matching
    DMA/gather arrival order.
  * ACT: single tanh over [32, 512] PSUM with per-partition bias; DVE
    scale; one output DMA.
"""

import math

import numpy as np

import concourse.bacc as bacc
import concourse.mybir as mybir
from concourse import tile
from concourse.bass_utils import run_bass_kernel_spmd

O, I, K, N = 32, 32, 13, 4096
NCORES = 8
NS = N // NCORES          # 512 local sites per core
NOG = O // 4              # 8 channel groups of 4
_E = math.e
SCALE = (2.0 + 2.0 * _E) / (_E - 1.0)

_F32 = mybir.dt.float32
_F16 = mybir.dt.float16
_I16 = mybir.dt.int16

_BUILT = {}


def _emit(nc, tc, d, pools):
    cpool, gpool, mpool, ppool, opool, qpool = pools

    idx_sb = cpool.tile([128, K * 32], _I16, tag="idx")
    nc.sync.dma_start(idx_sb[:, :], d["idx"][:, :])
    wt4f = cpool.tile([128, K, NOG, 4], _F32, tag="w4f")
    nc.scalar.dma_start(wt4f[:, :, :, :], d["wt4"][:, :, :, :])
    bcol_sb = cpool.tile([4, NOG], _F32, tag="bc")
    nc.scalar.dma_start(bcol_sb[:, :], d["bcol"][:, :])

    # g[32a+i, k, 0, n] = x[i, shifts[n, k]] via SWDGE gather from xT4 rows
    g = gpool.tile([128, K, 1, NS], _F16, tag="g")
    for k in range(K):
        nc.gpsimd.dma_gather(
            g[:, k, :, :],
            d["xT4"][:, :],
            idx_sb[:, 32 * k : 32 * (k + 1)],
            num_idxs=NS,
            num_idxs_reg=NS,
            elem_size=128,
            transpose=True,
            queue_num=k % 4,
        )

    # all 8 og mask tiles stay resident (13.3 KB/partition each); p-major
    # HBM layout gives 2-level-AP DMAs with contiguous per-partition lines.
    # Each og is split into two k-halves, all h0 halves DMA'd before any h1,
    # so every og's early taps are resident by the time its products start.
    KH = 7
    mts0, mts1 = [], []
    for og in range(NOG):
        mt0 = mpool.tile([128, KH, NS], _F16, tag=f"m{og}a", bufs=1, name=f"mt{og}a")
        eng = nc.sync if og % 2 == 0 else nc.scalar
        eng.dma_start(mt0[:, :, :], d["maskg"][og, :, 0:KH, :])
        mts0.append(mt0)
    for og in range(NOG):
        mt1 = mpool.tile(
            [128, K - KH, NS], _F16, tag=f"m{og}b", bufs=1, name=f"mt{og}b"
        )
        eng = nc.sync if og % 2 == 0 else nc.scalar
        eng.dma_start(mt1[:, :, :], d["maskg"][og, :, KH:K, :])
        mts1.append(mt1)

    def _mask_ap(og, k0, nb):
        if k0 + nb <= KH:
            return mts0[og][:, k0 : k0 + nb, :]
        return mts1[og][:, k0 - KH : k0 - KH + nb, :]

    wt4 = cpool.tile([128, K, NOG, 4], _F16, tag="w4")
    nc.vector.tensor_copy(wt4[:, :, :, :], wt4f[:, :, :, :])
    bhalf = opool.tile([4, NOG], _F32, tag="bh")
    nc.scalar.activation(
        bhalf[:, :], bcol_sb[:, :], mybir.ActivationFunctionType.Copy, scale=0.5
    )

    # one PSUM bank per og chain: [4, 512] f32 at base partition 0
    yps = [
        qpool.tile([4, NS], _F32, tag=f"yp{og}", bufs=1, name=f"yp{og}")
        for og in range(NOG)
    ]

    # PE warm-up: throwaway matmuls keep the HAM clock at 2.4 GHz through
    # the SWDGE-init dead time.  An initial burst (gated on a local memset)
    # plus batches gated on successive mask-tile arrivals to avoid the
    # idle-window re-throttle before the first products appear.
    warmsrc = cpool.tile([128, NS], _F16, tag="wsrc")
    nc.vector.memset(warmsrc[:, :], 0.0)
    for w in range(16):
        nc.tensor.matmul(
            yps[0][:, :], warmsrc[:, 0:4], warmsrc[:, :], start=True, stop=True
        )
    for og in range(4):
        for w in range(6):
            nc.tensor.matmul(
                yps[0][:, :], wt4[:, 0, og, :], mts0[og][:, 0, :],
                start=True, stop=True,
            )

    # Emission follows expected arrival: gathers land ~26+1.5k us, mask
    # halves land ~8/~28 + 4.6*(og//2) us; sort (kb, og) by the later of
    # the two.  DVE muls batch tap pairs (adjacent in both the mask tile
    # and the gather tile) to halve instruction overhead on the critical
    # DVE->PE lockstep.
    kbatches = [(0, 1), (2, 3), (4, 5), (6,), (7, 8), (9, 10), (11, 12)]

    def _key(kb, og):
        tg = 26.0 + 1.5 * kb[-1]
        tm = (8.0 if kb[-1] < KH else 28.0) + 4.6 * (og // 2)
        return (max(tg, tm), kb[-1], og)

    order = sorted(
        ((kb, og) for kb in kbatches for og in range(NOG)),
        key=lambda t: _key(t[0], t[1]),
    )
    for kb, og in order:
            k0, nb = kb[0], len(kb)
            prod = ppool.tile([128, nb, NS], _F16, tag=f"prod{nb}", bufs=6)
            nc.vector.tensor_mul(
                prod[:, :, :],
                _mask_ap(og, k0, nb),
                g[:, k0 : k0 + nb, 0, :],
            )
            for k in kb:
                nc.tensor.matmul(
                    yps[og][:, :],
                    wt4[:, k, og, :],
                    prod[:, k - k0, :],
                    start=(k == 0),
                    stop=(k == K - 1),
                )
            if kb[-1] == K - 1:
                ycat = opool.tile([4, NS], _F32, tag="ycat", bufs=2)
                nc.scalar.activation(
                    ycat[:, :], yps[og][:, :], mybir.ActivationFunctionType.Tanh,
                    bias=bhalf[:, og : og + 1], scale=0.5,
                )
                nc.vector.tensor_scalar_mul(ycat[:, :], ycat[:, :], SCALE / 2.0)
                eng = nc.sync if og % 2 == 0 else nc.scalar
                eng.dma_start(d["y"][4 * og : 4 * og + 4, :], ycat[:, :])


def _declare(nc):
    d = {}
    d["xT4"] = nc.declare_dram_parameter("xT4", [N, 128], _F16, isOutput=False)
    d["idx"] = nc.declare_dram_parameter("idx", [128, K * 32], _I16, isOutput=False)
    d["maskg"] = nc.declare_dram_parameter(
        "maskg", [NOG, 128, K, NS], _F16, isOutput=False
    )
    d["wt4"] = nc.declare_dram_parameter("wt4", [128, K, NOG, 4], _F32, isOutput=False)
    d["bcol"] = nc.declare_dram_parameter("bcol", [4, NOG], _F32, isOutput=False)
    d["y"] = nc.declare_dram_parameter("y", [O, NS], _F32, isOutput=True)
    return d


def _pools(tc, stack):
    names = [
        ("const", 1), ("gather", 1), ("mask", 8), ("prod", 6),
        ("out", 1), ("psum", 1),
    ]
    pools = []
    for name, bufs in names:
        kw = {"space": "PSUM"} if name == "psum" else {}
        pools.append(stack.enter_context(tc.tile_pool(name=name, bufs=bufs, **kw)))
    return pools


def _build():
    if "nc" in _BUILT:
        return _BUILT["nc"]
    from contextlib import ExitStack

    nc = bacc.Bacc(
        "TRN2", target_bir_lowering=False, debug=False, num_swdge_queues=4
    )
    d = _declare(nc)
    with tile.TileContext(nc) as tc:
        with ExitStack() as stack:
            pools = _pools(tc, stack)
            _emit(nc, tc, d, pools)
    nc.compile()
    _BUILT["nc"] = nc
    return nc


def _wrap16(col):
    """shifts column (NS,) -> (16, NS//16) wrapped layout: out[r, s] = col[s*16+r]."""
    return np.ascontiguousarray(col.reshape(NS // 16, 16).T)


def make_in_maps(x, Wconv, bconv, mask, shifts):
    """Host-side shard/layout prep: transposes/reshapes plus value-preserving
    dtype narrowing (f32 0/1 mask -> fp16 exactly; int32 indices < 4096 ->
    int16)."""
    x = np.asarray(x, dtype=np.float32)
    xT4 = np.ascontiguousarray(np.tile(x.T.astype(np.float16), (1, 4)))  # (N, 128)
    W = np.asarray(Wconv, dtype=np.float32)
    W4d = W.reshape(NOG, 4, I, K)                                # (og, j, i, k)
    wt4 = np.zeros((128, K, NOG, 4), np.float32)
    for j in range(4):
        wt4[32 * j : 32 * (j + 1), :, :, j] = W4d[:, j].transpose(1, 2, 0)
    bcol = np.ascontiguousarray(
        np.asarray(bconv, dtype=np.float32).reshape(NOG, 4).T
    )                                                            # (4, og)
    mask = np.asarray(mask)
    shifts = np.asarray(shifts)

    in_maps = []
    for core in range(NCORES):
        sl = slice(core * NS, (core + 1) * NS)
        m = mask[sl].astype(np.float16)                          # (NS, O, I, K)
        m = m.transpose(1, 2, 3, 0)                              # (O, I, K, NS)
        maskg = np.ascontiguousarray(m.reshape(NOG, 128, K, NS))
        sh = shifts[sl].astype(np.int16)                         # (NS, K)
        idx = np.empty((128, K * 32), np.int16)
        for k in range(K):
            w = _wrap16(sh[:, k])                                # (16, 32)
            for a in range(8):
                idx[16 * a : 16 * (a + 1), 32 * k : 32 * (k + 1)] = w
        in_maps.append(
            {"xT4": xT4, "idx": idx, "maskg": maskg, "wt4": wt4, "bcol": bcol}
        )
    return in_maps


def kernel(x, Wconv, bconv, mask, shifts):
    nc = _build()
    in_maps = make_in_maps(x, Wconv, bconv, mask, shifts)
    res = run_bass_kernel_spmd(nc, in_maps, core_ids=list(range(NCORES)))
    y = np.empty((O, N), np.float32)
    for core in range(NCORES):
        y[:, core * NS : (core + 1) * NS] = res.results[core]["y"]
    return y


# revision 29
# speedup vs baseline: 1.2812x; 1.0110x over previous
"""Trainium2 Bass kernel for the masked per-site stencil contraction

    y[o, n] = f( sum_{i,k} Wconv[o,i,k] * mask[n,o,i,k] * x[i, shifts[n,k]] + bconv[o] )
    f(v) = (sigmoid(v) - 0.5) * (2 + 2e)/(e - 1) = (2+2e)/(2(e-1)) * tanh(v/2)

Shapes: O=I=32, K=13, N=4096.  Sharded over 8 NeuronCores along the site
dimension N (512 sites per core); mask / shifts / output columns are
partitioned, x / Wconv / bconv replicated.

Per-core device plan (all cores run the identical SPMD program):
  * g built by 13 dma_gather calls (one per tap k) from xT4 in HBM, where
    xT4[s, 32a+i] = x[i, s] (x^T replicated 4x along features, 256B rows).
    transpose=True lands the feature dim on partitions: g_k[32a+i, n] =
    x[i, shifts[n, k]].  SWDGE descriptors prep on GPSIMD, data moves on
    the 16 DMA engines across 4 SWDGE queues.
  * mask shipped as fp16 (exact for a 0/1 mask) in [og, k, (j,i), n]
    layout: 4 output channels j packed along the 128-partition dim.
  * DVE: prod[(j,i), n] = mask_tile[og, k] * g_k  (fp16, 2x_1P mode)
  * PE:  one m=4 matmul per (og, k): ypsum[4og:4og+4, n] += W4^T @ prod,
    13-long accumulation chains per og, all 8 chains in ONE PSUM bank
    ([32, 512] f32).  (og, k) emission follows a diagonal sort matching
    DMA/gather arrival order.
  * ACT: single tanh over [32, 512] PSUM with per-partition bias; DVE
    scale; one output DMA.
"""

import math

import numpy as np

import concourse.bacc as bacc
import concourse.mybir as mybir
from concourse import tile
from concourse.bass_utils import run_bass_kernel_spmd

O, I, K, N = 32, 32, 13, 4096
NCORES = 8
NS = N // NCORES          # 512 local sites per core
NOG = O // 4              # 8 channel groups of 4
_E = math.e
SCALE = (2.0 + 2.0 * _E) / (_E - 1.0)

_F32 = mybir.dt.float32
_F16 = mybir.dt.float16
_I16 = mybir.dt.int16

_BUILT = {}


def _emit(nc, tc, d, pools):
    cpool, gpool, mpool, ppool, opool, qpool = pools

    idx_sb = cpool.tile([128, K * 32], _I16, tag="idx")
    nc.sync.dma_start(idx_sb[:, :], d["idx"][:, :])
    wt4f = cpool.tile([128, K, NOG, 4], _F32, tag="w4f")
    nc.scalar.dma_start(wt4f[:, :, :, :], d["wt4"][:, :, :, :])
    bcol_sb = cpool.tile([4, NOG], _F32, tag="bc")
    nc.scalar.dma_start(bcol_sb[:, :], d["bcol"][:, :])

    # g[32a+i, k, 0, n] = x[i, shifts[n, k]] via SWDGE gather from xT4 rows
    g = gpool.tile([128, K, 1, NS], _F16, tag="g")
    nreg = nc.gpsimd.to_reg(NS)
    for k in range(K):
        nc.gpsimd.dma_gather(
            g[:, k, :, :],
            d["xT4"][:, :],
            idx_sb[:, 32 * k : 32 * (k + 1)],
            num_idxs=NS,
            num_idxs_reg=nreg,
            elem_size=128,
            transpose=True,
            queue_num=k % 4,
        )

    # all 8 og mask tiles stay resident (13.3 KB/partition each); p-major
    # HBM layout gives 2-level-AP DMAs with contiguous per-partition lines.
    # Each og is split into two k-halves, all h0 halves DMA'd before any h1,
    # so every og's early taps are resident by the time its products start.
    KH = 7
    mts0, mts1 = [], []
    for og in range(NOG):
        mt0 = mpool.tile([128, KH, NS], _F16, tag=f"m{og}a", bufs=1, name=f"mt{og}a")
        eng = nc.sync if og % 2 == 0 else nc.scalar
        eng.dma_start(mt0[:, :, :], d["maskg"][og, :, 0:KH, :])
        mts0.append(mt0)
    for og in range(NOG):
        mt1 = mpool.tile(
            [128, K - KH, NS], _F16, tag=f"m{og}b", bufs=1, name=f"mt{og}b"
        )
        eng = nc.sync if og % 2 == 0 else nc.scalar
        eng.dma_start(mt1[:, :, :], d["maskg"][og, :, KH:K, :])
        mts1.append(mt1)

    def _mask_ap(og, k0, nb):
        if k0 + nb <= KH:
            return mts0[og][:, k0 : k0 + nb, :]
        return mts1[og][:, k0 - KH : k0 - KH + nb, :]

    wt4 = cpool.tile([128, K, NOG, 4], _F16, tag="w4")
    nc.vector.tensor_copy(wt4[:, :, :, :], wt4f[:, :, :, :])
    bhalf = opool.tile([4, NOG], _F32, tag="bh")
    nc.scalar.activation(
        bhalf[:, :], bcol_sb[:, :], mybir.ActivationFunctionType.Copy, scale=0.5
    )

    # one PSUM bank per og chain: [4, 512] f32 at base partition 0
    yps = [
        qpool.tile([4, NS], _F32, tag=f"yp{og}", bufs=1, name=f"yp{og}")
        for og in range(NOG)
    ]

    # PE warm-up: throwaway matmuls keep the HAM clock at 2.4 GHz through
    # the SWDGE-init dead time.  An initial burst (gated on a local memset)
    # plus batches gated on successive mask-tile arrivals to avoid the
    # idle-window re-throttle before the first products appear.
    warmsrc = cpool.tile([128, NS], _F16, tag="wsrc")
    nc.vector.memset(warmsrc[:, :], 0.0)
    for w in range(16):
        nc.tensor.matmul(
            yps[0][:, :], warmsrc[:, 0:4], warmsrc[:, :], start=True, stop=True
        )
    for og in range(4):
        for w in range(6):
            nc.tensor.matmul(
                yps[0][:, :], wt4[:, 0, og, :], mts0[og][:, 0, :],
                start=True, stop=True,
            )

    # Emission follows expected arrival: gathers land ~26+1.5k us, mask
    # halves land ~8/~28 + 4.6*(og//2) us; sort (kb, og) by the later of
    # the two.  DVE muls batch tap pairs (adjacent in both the mask tile
    # and the gather tile) to halve instruction overhead on the critical
    # DVE->PE lockstep.
    kbatches = [(0, 1), (2, 3), (4, 5), (6,), (7, 8), (9, 10), (11, 12)]

    def _key(kb, og):
        tg = 26.0 + 1.5 * kb[-1]
        tm = (8.0 if kb[-1] < KH else 28.0) + 4.6 * (og // 2)
        return (max(tg, tm), kb[-1], og)

    order = sorted(
        ((kb, og) for kb in kbatches for og in range(NOG)),
        key=lambda t: _key(t[0], t[1]),
    )
    for kb, og in order:
            k0, nb = kb[0], len(kb)
            prod = ppool.tile([128, nb, NS], _F16, tag=f"prod{nb}", bufs=6)
            nc.vector.tensor_mul(
                prod[:, :, :],
                _mask_ap(og, k0, nb),
                g[:, k0 : k0 + nb, 0, :],
            )
            for k in kb:
                nc.tensor.matmul(
                    yps[og][:, :],
                    wt4[:, k, og, :],
                    prod[:, k - k0, :],
                    start=(k == 0),
                    stop=(k == K - 1),
                )
            if kb[-1] == K - 1:
                ycat = opool.tile([4, NS], _F32, tag="ycat", bufs=2)
                nc.scalar.activation(
                    ycat[:, :], yps[og][:, :], mybir.ActivationFunctionType.Tanh,
                    bias=bhalf[:, og : og + 1], scale=0.5,
                )
                nc.vector.tensor_scalar_mul(ycat[:, :], ycat[:, :], SCALE / 2.0)
                eng = nc.sync if og % 2 == 0 else nc.scalar
                eng.dma_start(d["y"][4 * og : 4 * og + 4, :], ycat[:, :])


def _declare(nc):
    d = {}
    d["xT4"] = nc.declare_dram_parameter("xT4", [N, 128], _F16, isOutput=False)
    d["idx"] = nc.declare_dram_parameter("idx", [128, K * 32], _I16, isOutput=False)
    d["maskg"] = nc.declare_dram_parameter(
        "maskg", [NOG, 128, K, NS], _F16, isOutput=False
    )
    d["wt4"] = nc.declare_dram_parameter("wt4", [128, K, NOG, 4], _F32, isOutput=False)
    d["bcol"] = nc.declare_dram_parameter("bcol", [4, NOG], _F32, isOutput=False)
    d["y"] = nc.declare_dram_parameter("y", [O, NS], _F32, isOutput=True)
    return d


def _pools(tc, stack):
    names = [
        ("const", 1), ("gather", 1), ("mask", 8), ("prod", 6),
        ("out", 1), ("psum", 1),
    ]
    pools = []
    for name, bufs in names:
        kw = {"space": "PSUM"} if name == "psum" else {}
        pools.append(stack.enter_context(tc.tile_pool(name=name, bufs=bufs, **kw)))
    return pools


def _build():
    if "nc" in _BUILT:
        return _BUILT["nc"]
    from contextlib import ExitStack

    nc = bacc.Bacc(
        "TRN2", target_bir_lowering=False, debug=False, num_swdge_queues=4
    )
    d = _declare(nc)
    with tile.TileContext(nc) as tc:
        with ExitStack() as stack:
            pools = _pools(tc, stack)
            _emit(nc, tc, d, pools)
    nc.compile()
    _BUILT["nc"] = nc
    return nc


def _wrap16(col):
    """shifts column (NS,) -> (16, NS//16) wrapped layout: out[r, s] = col[s*16+r]."""
    return np.ascontiguousarray(col.reshape(NS // 16, 16).T)


def make_in_maps(x, Wconv, bconv, mask, shifts):
    """Host-side shard/layout prep: transposes/reshapes plus value-preserving
    dtype narrowing (f32 0/1 mask -> fp16 exactly; int32 indices < 4096 ->
    int16)."""
    x = np.asarray(x, dtype=np.float32)
    xT4 = np.ascontiguousarray(np.tile(x.T.astype(np.float16), (1, 4)))  # (N, 128)
    W = np.asarray(Wconv, dtype=np.float32)
    W4d = W.reshape(NOG, 4, I, K)                                # (og, j, i, k)
    wt4 = np.zeros((128, K, NOG, 4), np.float32)
    for j in range(4):
        wt4[32 * j : 32 * (j + 1), :, :, j] = W4d[:, j].transpose(1, 2, 0)
    bcol = np.ascontiguousarray(
        np.asarray(bconv, dtype=np.float32).reshape(NOG, 4).T
    )                                                            # (4, og)
    mask = np.asarray(mask)
    shifts = np.asarray(shifts)

    in_maps = []
    for core in range(NCORES):
        sl = slice(core * NS, (core + 1) * NS)
        m = mask[sl].astype(np.float16)                          # (NS, O, I, K)
        m = m.transpose(1, 2, 3, 0)                              # (O, I, K, NS)
        maskg = np.ascontiguousarray(m.reshape(NOG, 128, K, NS))
        sh = shifts[sl].astype(np.int16)                         # (NS, K)
        idx = np.empty((128, K * 32), np.int16)
        for k in range(K):
            w = _wrap16(sh[:, k])                                # (16, 32)
            for a in range(8):
                idx[16 * a : 16 * (a + 1), 32 * k : 32 * (k + 1)] = w
        in_maps.append(
            {"xT4": xT4, "idx": idx, "maskg": maskg, "wt4": wt4, "bcol": bcol}
        )
    return in_maps


def kernel(x, Wconv, bconv, mask, shifts):
    nc = _build()
    in_maps = make_in_maps(x, Wconv, bconv, mask, shifts)
    res = run_bass_kernel_spmd(nc, in_maps, core_ids=list(range(NCORES)))
    y = np.empty((O, N), np.float32)
    for core in range(NCORES):
        y[:, core * NS : (core + 1) * NS] = res.results[core]["y"]
    return y
